# revision 1
# baseline (speedup 1.0000x reference)
"""DLSTMCell hypernetwork kernel for Trainium2 (runs on 4 of 8 NeuronCores).

Reference computation (per stock n of N=2048):
    mem  = emb_table[index]                       (N, 128)
    h1   = tanh(mem @ w1.T + b1)                  (N, 128)
    h    = tanh(h1 @ w2.T + b2)                   (N, 64)
    W_n  = (h @ w3.T + b3).reshape(192, 512)      per-stock LSTM weights
    z    = data_n @ W_n + lstm_bias               data = [x, hx]  (192,)
    g    = sigmoid(z); i,f,gg,o = split(g)
    cy   = cx*sigmoid(f) + sigmoid(i)*tanh(gg)
    hy   = sigmoid(o)*tanh(cy)

Key fusion: the (N, 192, 512) = 805MB weights tensor is never materialized.
    z[n,k] = sum_{d,b} (data[n,d]*h[n,b]) * W3perm[(d,b),k] + sum_d B3r[d,k]*data[n,d]
is a standard dense matmul with the SHARED (12288, 512) matrix W3perm against
per-stock outer-product tiles opT[(d,b), n], accumulated in PSUM.

Sharding: data-parallel over stocks on 4 cores (512 each). The 8 visible
cores oversubscribe the device: >4 concurrent cores measured ~2x slower
per core, so 4 cores give the best wall time. W3perm is replicated.

Precision: W3perm and the outer-product tiles are bf16 (measured end-to-end
rel err 6e-6 — indistinguishable from f32 here because the hypernetwork
matmul term is small against lstm_bias and two sigmoids compress errors);
everything else f32/f32r. PSUM accumulation is always f32.

Layout: gates kept transposed [k, n] so the gate unit k sits on partitions:
lstm_bias folds into the ACT sigmoid as a per-partition bias and the LSTM
epilogue runs on [128, n] tiles.

opT construction on-device, pipelined LA pair-units ahead of the gate
matmuls: A_t = rows (2t, 2t+1) of dataT each replicated 64x, built by a K=2
matmul against a constant 0/1 pattern (PE broadcast, writes PSUM);
opT = A * [hT; hT] on the vector engine, two K-tiles per DVE op.
"""
import sys

sys.path.insert(0, "/opt/trn_rl_repo")
import numpy as np
import ml_dtypes
import concourse.bacc as bacc
import concourse.mybir as mybir
import concourse.tile as tile
from concourse.bass_utils import run_bass_kernel_spmd

F32 = mybir.dt.float32
F32R = mybir.dt.float32r
BF16 = mybir.dt.bfloat16
FP8 = mybir.dt.float8e4
AF = mybir.ActivationFunctionType

USE_FP8 = False              # fp8e4 DoubleRow gate matmuls (2 K-tiles/matmul)
OP_SCALE = 8.0              # folded into the ppat broadcast constant
W3_SCALE = 64.0             # keeps w3 (~0.02 sigma) in e4m3 normal range
Z_DESCALE = 1.0 / (OP_SCALE * W3_SCALE)

N = 2048
INPUT = 64
EMB = 128
BOT = 64
HID = 128
WDIM = 4 * (INPUT + HID) * HID
NCORES = 4
NC_N = N // NCORES          # 512 stocks per core
D = INPUT + HID             # 192
K4 = 4 * HID                # 512 gate columns
KT = (D * BOT) // 128       # 96 contraction K-tiles of 128
KU = KT // 2                # 48 paired K-tiles

_cache = {}


def _build_program(repeat=1):
    """repeat>1 wraps the compute body in a hardware loop — used only for
    wall-clock slope timing (exec_ns ~= (wall[R2]-wall[R1])/(R2-R1))."""
    nc = bacc.Bacc(None)

    di = lambda name, shape, dt: nc.dram_tensor(name, shape, dt, kind="ExternalInput")
    memT_d = di("memT", [EMB, NC_N], F32R)
    dT0_d = di("dT0", [INPUT, NC_N], F32R)
    dT1_d = di("dT1", [HID, NC_N], F32R)
    # data row pairs packed 3-per-free-window at base partitions {0,32,64}:
    # pair t -> dPair3[32*(t%3) : +2, (t//3)*NC_N : (t//3+1)*NC_N].
    # DRAM carries only the 3 used row-pairs ([6, ...]); the zero rows of
    # the SBUF tile are never read so they need no DMA.
    dPair3_d = di("dPair3", [6, (KT // 3) * NC_N], F32R)
    cxT_d = di("cxT", [HID, NC_N], F32)
    w1T_d = di("w1T", [EMB, EMB], F32R)
    w2T_d = di("w2T", [EMB, BOT], F32R)
    b1_d = di("b1c", [EMB, 1], F32)
    b2_d = di("b2c", [BOT, 1], F32)
    # W3perm paired: rows 128u.. hold K-tiles 2u and 2u+1. bf16: side by
    # side [2u | 2u+1]; fp8 DoubleRow: interleaved [K, 2, k] with the pair
    # on the middle axis.
    w3p_d = di("w3p", [KU * 128, 2 * K4], FP8 if USE_FP8 else BF16)
    b3a_d = di("b3a", [INPUT, K4], F32R)
    b3b_d = di("b3b", [HID, K4], F32R)
    lb_d = di("lbias", [HID, 4], F32)
    ppat_d = di("ppat", [66, 128], F32R)
    hyT_o = nc.dram_tensor("hyT", [HID, NC_N], F32, kind="ExternalOutput")
    cyT_o = nc.dram_tensor("cyT", [HID, NC_N], F32, kind="ExternalOutput")

    with tile.TileContext(nc) as tc:
        with tc.tile_pool(name="const", bufs=1) as const, \
             tc.tile_pool(name="w3", bufs=6) as w3pool, \
             tc.tile_pool(name="op", bufs=4) as opool, \
             tc.tile_pool(name="ep", bufs=1) as ep, \
             tc.tile_pool(name="psA", bufs=2, space="PSUM") as psA, \
             tc.tile_pool(name="psG", bufs=1, space="PSUM") as psG:

            def load(dram, shape, dt, tag=None):
                nm = tag or dram.name
                t = const.tile(shape, dt, tag=nm, name=nm)
                nc.sync.dma_start(t[:], dram[:])
                return t

            memT = load(memT_d, [EMB, NC_N], F32R)
            dT0 = load(dT0_d, [INPUT, NC_N], F32R)
            dT1 = load(dT1_d, [HID, NC_N], F32R)
            dPair3 = const.tile([66, (KT // 3) * NC_N], F32R, name="dPair3")
            for gp in range(3):
                nc.sync.dma_start(
                    dPair3[32 * gp:32 * gp + 2, :], dPair3_d[2 * gp:2 * gp + 2, :]
                )
            cxT = load(cxT_d, [HID, NC_N], F32)
            w1T = load(w1T_d, [EMB, EMB], F32R)
            w2T = load(w2T_d, [EMB, BOT], F32R)
            b1c = load(b1_d, [EMB, 1], F32)
            b2c = load(b2_d, [BOT, 1], F32)
            b3a = load(b3a_d, [INPUT, K4], F32R)
            b3b = load(b3b_d, [HID, K4], F32R)
            lb = load(lb_d, [HID, 4], F32)
            ppat = load(ppat_d, [66, 128], F32R)

            from contextlib import ExitStack
            loop_ctx = ExitStack()
            if repeat > 1:
                loop_ctx.enter_context(
                    tc.For_i(0, repeat, 1, hint_engines=(mybir.EngineType.PE,))
                )

            # gate accumulators [k-chunk, n] — 4 full PSUM banks
            psg = [
                psG.tile([128, NC_N], F32, tag=f"g{kc}", name=f"psg{kc}")
                for kc in range(4)
            ]

            # hypernetwork MLP (PSUM scratch borrowed from psg banks; the
            # later start=True bias matmuls reset them for accumulation)
            nc.tensor.matmul(psg[0][:], w1T[:], memT[:], start=True, stop=True)
            h1T = ep.tile([128, NC_N], F32R, tag="h1T")
            nc.scalar.activation(h1T[:], psg[0][:], AF.Tanh, bias=b1c[:])
            nc.tensor.matmul(psg[1][0:BOT, :], w2T[:], h1T[:], start=True, stop=True)
            hT2 = ep.tile([128, NC_N], F32R, tag="hT2")
            nc.scalar.activation(hT2[0:BOT, :], psg[1][0:BOT, :], AF.Tanh, bias=b2c[:])
            nc.scalar.activation(hT2[BOT:128, :], psg[1][0:BOT, :], AF.Tanh, bias=b2c[:])

            # fold the b3 term in first (start=True resets the banks)
            for kc in range(4):
                ks = slice(kc * 128, kc * 128 + 128)
                nc.tensor.matmul(psg[kc][:], b3a[:, ks], dT0[:], start=True, stop=False)
                nc.tensor.matmul(psg[kc][:], b3b[:, ks], dT1[:], start=False, stop=False)

            # main contraction: 48 pair-units u = K-tiles (2u, 2u+1).
            # Stage A (per u): two K=2 broadcast matmuls -> pa2 [128,1024] PSUM,
            # one DVE mul -> op2 [128,2,512] bf16, one 512KB W3 DMA.
            # Gate matmuls consume pair u LA units later.
            LA = 2
            op_q = []
            w3_q = []

            def emit_stage_a(u):
                w3sb = w3pool.tile(
                    [128, 2, K4] if USE_FP8 else [128, 2 * K4],
                    FP8 if USE_FP8 else BF16, tag="w3sb", name="w3sb")
                src = w3p_d[u * 128:(u + 1) * 128, :]
                if USE_FP8:
                    src = src.rearrange("p (h k) -> p h k", h=2)
                nc.sync.dma_start(w3sb[:], src)
                w3_q.append(w3sb)
                pa2 = psA.tile([128, 2 * NC_N], F32, tag="A", name="pa2")
                for h in range(2):
                    t = 2 * u + h
                    gp, slot = t % 3, t // 3
                    nc.tensor.matmul(
                        pa2[:, h * NC_N:(h + 1) * NC_N],
                        ppat[32 * gp:32 * gp + 2, :],
                        dPair3[32 * gp:32 * gp + 2, slot * NC_N:(slot + 1) * NC_N],
                        start=True, stop=True,
                    )
                op2 = opool.tile([128, 2, NC_N], FP8 if USE_FP8 else BF16,
                                 tag="opT", name="op2")
                nc.vector.tensor_mul(
                    op2[:],
                    pa2[:].rearrange("p (h n) -> p h n", h=2),
                    hT2[:, None, :].broadcast_to([128, 2, NC_N]),
                )
                op_q.append(op2)

            for u in range(min(LA, KU)):
                emit_stage_a(u)
            for u in range(KU):
                if u + LA < KU:
                    emit_stage_a(u + LA)
                last = u == KU - 1
                if USE_FP8:
                    for kc in range(4):
                        nc.tensor.matmul(
                            psg[kc][:],
                            w3_q[u][:, :, kc * 128:kc * 128 + 128],
                            op_q[u][:],
                            start=False, stop=last,
                            perf_mode=mybir.MatmulPerfMode.DoubleRow,
                        )
                else:
                    for h in range(2):
                        for kc in range(4):
                            nc.tensor.matmul(
                                psg[kc][:],
                                w3_q[u][:, h * K4 + kc * 128:h * K4 + kc * 128 + 128],
                                op_q[u][:, h, :],
                                start=False, stop=last and h == 1,
                            )
                w3_q[u] = op_q[u] = None

            # LSTM epilogue on [hid, n] tiles; k-chunk order: i, f, g, o
            g = []
            for kc in range(4):
                gt = ep.tile([128, NC_N], F32, tag=f"gs{kc}", name=f"gs{kc}")
                nc.scalar.activation(gt[:], psg[kc][:], AF.Sigmoid,
                                     bias=lb[:, kc:kc + 1],
                                     scale=Z_DESCALE if USE_FP8 else 1.0)
                g.append(gt)
            i_t = ep.tile([128, NC_N], F32, tag="i_t")
            nc.scalar.activation(i_t[:], g[0][:], AF.Sigmoid)
            f_t = ep.tile([128, NC_N], F32, tag="f_t")
            nc.scalar.activation(f_t[:], g[1][:], AF.Sigmoid)
            g_t = ep.tile([128, NC_N], F32, tag="g_t")
            nc.scalar.activation(g_t[:], g[2][:], AF.Tanh)
            o_t = ep.tile([128, NC_N], F32, tag="o_t")
            nc.scalar.activation(o_t[:], g[3][:], AF.Sigmoid)

            t1 = ep.tile([128, NC_N], F32, tag="t1")
            nc.vector.tensor_mul(t1[:], cxT[:], f_t[:])
            t2 = ep.tile([128, NC_N], F32, tag="t2")
            nc.vector.tensor_mul(t2[:], i_t[:], g_t[:])
            cy = ep.tile([128, NC_N], F32, tag="cy")
            nc.vector.tensor_add(cy[:], t1[:], t2[:])
            tcy = ep.tile([128, NC_N], F32, tag="tcy")
            nc.scalar.activation(tcy[:], cy[:], AF.Tanh)
            hy = ep.tile([128, NC_N], F32, tag="hy")
            nc.vector.tensor_mul(hy[:], o_t[:], tcy[:])

            nc.sync.dma_start(cyT_o[:], cy[:])
            nc.sync.dma_start(hyT_o[:], hy[:])

            loop_ctx.close()

    nc.finalize()
    return nc


def kernel(x, index, hx, cx, emb_table, w1, b1, w2, b2, w3, b3, lstm_bias,
           _trace=False):
    x = np.asarray(x, np.float32)
    index = np.asarray(index)
    hx = np.asarray(hx, np.float32)
    cx = np.asarray(cx, np.float32)
    emb_table = np.asarray(emb_table, np.float32)
    w1 = np.asarray(w1, np.float32)
    b1 = np.asarray(b1, np.float32)
    w2 = np.asarray(w2, np.float32)
    b2 = np.asarray(b2, np.float32)
    w3 = np.asarray(w3, np.float32)
    b3 = np.asarray(b3, np.float32)
    lstm_bias = np.asarray(lstm_bias, np.float32)

    if "nc" not in _cache:
        _cache["nc"] = _build_program()
    nc = _cache["nc"]

    # host-side input prep (sharding + layout)
    mem = emb_table[index]                                   # (N, EMB)
    c = np.ascontiguousarray
    w1T = c(w1.T)
    w2T = c(w2.T)
    b1c = b1.reshape(EMB, 1)
    b2c = b2.reshape(BOT, 1)
    # W3perm[(d*64+b), k] = w3[d*512+k, b]; then pair K-tiles (2u, 2u+1)
    w3perm = w3.reshape(D, K4, BOT).transpose(0, 2, 1).reshape(D * BOT, K4)
    w3pair = w3perm.reshape(KU, 2, 128, K4).transpose(0, 2, 1, 3)
    if USE_FP8:
        w3p = c((w3pair * W3_SCALE).reshape(KU * 128, 2 * K4)).astype(
            ml_dtypes.float8_e4m3)
        zs = OP_SCALE * W3_SCALE
    else:
        w3p = c(w3pair.reshape(KU * 128, 2 * K4)).astype(ml_dtypes.bfloat16)
        zs = 1.0
    b3r = b3.reshape(D, K4) * zs
    b3a = c(b3r[0:INPUT])
    b3b = c(b3r[INPUT:D])
    lbias = c(lstm_bias.reshape(4, HID).T)                   # [j, kc]
    ppat = np.zeros((66, 128), np.float32)
    pv = OP_SCALE if USE_FP8 else 1.0
    for gp in range(3):
        ppat[32 * gp, 0:64] = pv
        ppat[32 * gp + 1, 64:128] = pv

    in_maps = []
    for ci in range(NCORES):
        sl = slice(ci * NC_N, (ci + 1) * NC_N)
        dataT = np.concatenate([x[sl].T, hx[sl].T], axis=0)  # (192, NC_N)
        dp3 = np.zeros((6, (KT // 3) * NC_N), np.float32)
        for t in range(KT):
            gp, slot = t % 3, t // 3
            dp3[2 * gp:2 * gp + 2, slot * NC_N:(slot + 1) * NC_N] = \
                dataT[2 * t:2 * t + 2]
        in_maps.append({
            "memT": c(mem[sl].T),
            "dT0": c(x[sl].T),
            "dT1": c(hx[sl].T),
            "dPair3": dp3,
            "cxT": c(cx[sl].T),
            "w1T": w1T, "w2T": w2T, "b1c": b1c, "b2c": b2c,
            "w3p": w3p, "b3a": b3a, "b3b": b3b,
            "lbias": lbias, "ppat": ppat,
        })

    res = run_bass_kernel_spmd(nc, in_maps, list(range(NCORES)), trace=_trace)
    hy = np.concatenate([r["hyT"].T for r in res.results], axis=0)
    cy = np.concatenate([r["cyT"].T for r in res.results], axis=0)
    if _trace:
        kernel.last_results = res
    return hy.astype(np.float32), cy.astype(np.float32)



# revision 15
# speedup vs baseline: 7.5380x; 7.5380x over previous
"""DLSTMCell hypernetwork kernel for Trainium2 (data-parallel on NCORES cores).

Reference computation (per stock n of N=2048):
    mem  = emb_table[index]                       (N, 128)
    h1   = tanh(mem @ w1.T + b1)                  (N, 128)
    h    = tanh(h1 @ w2.T + b2)                   (N, 64)
    W_n  = (h @ w3.T + b3).reshape(192, 512)      per-stock LSTM weights
    z    = data_n @ W_n + lstm_bias               data = [x, hx]  (192,)
    g    = sigmoid(z); i,f,gg,o = split(g)
    cy   = cx*sigmoid(f) + sigmoid(i)*tanh(gg)
    hy   = sigmoid(o)*tanh(cy)

Key fusion: the (N, 192, 512) = 805MB weights tensor is never materialized.
    z[n,k] = sum_{d,b} (data[n,d]*h[n,b]) * W3perm[(d,b),k] + sum_d B3r[d,k]*data[n,d]
is a dense matmul of the SHARED (12288, 512) matrix W3perm against per-stock
outer-product tiles opT[(d,b), n], accumulated in PSUM.

Measured engine rates (this silicon, loop-slope microbenchmarks):
  - fp8e4 DoubleRow gate matmul @ FD=512: ~220 ns/MM = true 2x over bf16.
  - K<=32 broadcast matmuls run ~3x-concurrent across the four 32-row PE
    strips (explicit tile_position) when emitted back-to-back.
  - DVE tensor_mul PSUM-f32 x f32r -> fp8 is 1x mode, ~1.03 us per
    [128,2,512] unit (PSUM source caps the mode; DVE fp8 *output* from
    SBUF bf16 sources is even slower, so PSUM->fp8 direct is optimal).
  - dma_start costs ~1.2 us of issuing-engine time regardless of size;
    single queue ~234 GB/s. w3 therefore streams as 12 x 512KB transfers
    alternating between the two HWDGE queues (SP via nc.sync, ACT via
    nc.scalar), and the broadcast operand is built on-device instead of
    DMA-ing a repacked layout.

Schedule: 48 pair-units (2 K-tiles each), blocks of BLK=2 units. Per block:
4 back-to-back K=32 strip-selector broadcast matmuls (a one-hot stationary
window of `ppat` picks data rows (2t, 2t+1) out of an aligned 32-row strip
of dT0/dT1 — BIR requires 32-aligned matmul partition bases) -> pa PSUM,
rotating strips per PAIR_ORDER for row-tile concurrency (host permutes w3
to match, contraction order is free); then BLK DVE muls with hT2 -> op fp8;
then (LA_B blocks behind) 4 DoubleRow gate matmuls per unit. The LSTM
epilogue runs in two half-n chunks so its ACT and DVE chains pipeline.

Precision: w3 and op tiles are fp8e4m3 with power-of-two scales folded into
the broadcast pattern / w3 and descaled in the gate sigmoid. The
hypernetwork matmul term is tiny against lstm_bias (dropping it entirely
would only move the output by ~1.6e-3 relative), so fp8 quantization of
this term is far inside the 2e-2 gate; measured end-to-end rel err ~6e-5.

Sharding: data-parallel over stocks on NCORES cores; cores execute in
parallel (loop-slope at fixed per-core work: 1 core 119us/iter, 8 cores
169us/iter)."""
import sys

sys.path.insert(0, "/opt/trn_rl_repo")
import numpy as np
import ml_dtypes
import concourse.bacc as bacc
import concourse.mybir as mybir
import concourse.tile as tile
from concourse.bass_utils import run_bass_kernel_spmd

F32 = mybir.dt.float32
F32R = mybir.dt.float32r
BF16 = mybir.dt.bfloat16
FP8 = mybir.dt.float8e4
AF = mybir.ActivationFunctionType
DRMODE = mybir.MatmulPerfMode.DoubleRow

USE_FP8 = True
OP_SCALE = 32.0             # folded into the ppat broadcast constant
W3_SCALE = 64.0             # keeps w3 (~0.02 sigma) in e4m3 normal range
Z_DESCALE = 1.0 / (OP_SCALE * W3_SCALE)

N = 2048
INPUT = 64
EMB = 128
BOT = 64
HID = 128
WDIM = 4 * (INPUT + HID) * HID
NCORES = 4                  # data-parallel cores (4 or 8 both validated)
NC_N = N // NCORES          # stocks per core
D = INPUT + HID             # 192
K4 = 4 * HID                # 512 gate columns
KT = (D * BOT) // 128       # 96 contraction K-tiles of 128
KU = KT // 2                # 48 paired K-tiles
BLK = 2                     # pair-units per schedule block
DBLK = 4                    # pair-units per w3 DMA (512KB fp8 transfers)
LA_B = 2                    # block lookahead (stage A runs LA_B blocks ahead)


def _pair_order():
    """Schedule the 96 K-tile pairs so consecutive pa matmuls hit
    different 32-row PE strips (enables row-tile concurrency).
    Strip of pair t: rows (2t,2t+1) of dT0 (t<32) or dT1 (t>=32)."""
    pools = {0: [], 1: [], 2: [], 3: []}
    for t in range(KT):
        base = 2 * t if t < 32 else 2 * (t - 32)
        pools[(base % 128) // 32].append(t)
    order = []
    for rep in range(16):
        for s in (0, 1, 2, 0, 1, 3):
            order.append(pools[s].pop(0))
    assert sorted(order) == list(range(KT))
    return order


PAIR_ORDER = _pair_order()

# pipeline-depth knobs (functions of per-core n)
PSA_BUFS = lambda nc_n: 2 if nc_n > 256 else 4
LAB_F = lambda nc_n: LA_B if nc_n > 256 else LA_B + 1

# ablation switches (timing experiments only — leave False for real runs)
SKIP_GATES = False          # drop gate matmuls
SKIP_MULS = False           # feed gates from a constant op tile, no DVE muls
SKIP_PA = False             # drop pa broadcast matmuls (implies SKIP_MULS)

_cache = {}


def _build_program(repeat=1, nc_n=None):
    """repeat>1 wraps the compute body in a hardware loop — used only for
    wall-clock slope timing (exec_ns ~= (wall[R2]-wall[R1])/(R2-R1))."""
    nc_n = nc_n or (N // NCORES)
    nc = bacc.Bacc(None)

    di = lambda name, shape, dt: nc.dram_tensor(name, shape, dt, kind="ExternalInput")
    memT_d = di("memT", [EMB, nc_n], F32R)
    dT0_d = di("dT0", [INPUT, nc_n], F32R)
    dT1_d = di("dT1", [HID, nc_n], F32R)
    cxT_d = di("cxT", [HID, nc_n], F32)
    w1T_d = di("w1T", [EMB, EMB], F32R)
    w2T_d = di("w2T", [EMB, BOT], F32R)
    b1_d = di("b1c", [EMB, 1], F32)
    b2_d = di("b2c", [BOT, 1], F32)
    # W3perm paired: rows 128u.. hold K-tiles 2u and 2u+1. bf16: side by
    # side [2u | 2u+1]; fp8 DoubleRow: interleaved [K, 2, k] with the pair
    # on the middle axis.
    w3p_d = di("w3p", [128, KU * 2 * K4], FP8 if USE_FP8 else BF16)
    b3a_d = di("b3a", [INPUT, K4], F32R)
    b3b_d = di("b3b", [HID, K4], F32R)
    lb_d = di("lbias", [HID, 4], F32)
    ppat_d = di("ppat", [128, 16 * 128], F32R)
    hyT_o = nc.dram_tensor("hyT", [HID, nc_n], F32, kind="ExternalOutput")
    cyT_o = nc.dram_tensor("cyT", [HID, nc_n], F32, kind="ExternalOutput")

    with tile.TileContext(nc) as tc:
        with tc.tile_pool(name="const", bufs=1) as const, \
             tc.tile_pool(name="w3", bufs=3) as w3pool, \
             tc.tile_pool(name="op", bufs=6) as opool, \
             tc.tile_pool(name="ep", bufs=1) as ep, \
             tc.tile_pool(name="psA", bufs=PSA_BUFS(nc_n),
                          space="PSUM") as psA, \
             tc.tile_pool(name="psG", bufs=1, space="PSUM") as psG:

            _ldq = [0]

            def load(dram, shape, dt, tag=None):
                nm = tag or dram.name
                t = const.tile(shape, dt, tag=nm, name=nm)
                q = nc.sync if _ldq[0] % 2 == 0 else nc.scalar
                _ldq[0] += 1
                q.dma_start(t[:], dram[:])
                return t

            memT = load(memT_d, [EMB, nc_n], F32R)
            dT0 = load(dT0_d, [INPUT, nc_n], F32R)
            dT1 = load(dT1_d, [HID, nc_n], F32R)
            cxT = load(cxT_d, [HID, nc_n], F32)
            w1T = load(w1T_d, [EMB, EMB], F32R)
            w2T = load(w2T_d, [EMB, BOT], F32R)
            b1c = load(b1_d, [EMB, 1], F32)
            b2c = load(b2_d, [BOT, 1], F32)
            b3a = load(b3a_d, [INPUT, K4], F32R)
            b3b = load(b3b_d, [HID, K4], F32R)
            lb = load(lb_d, [HID, 4], F32)
            ppat = load(ppat_d, [128, 16 * 128], F32R)

            from contextlib import ExitStack
            loop_ctx = ExitStack()
            if repeat > 1:
                loop_ctx.enter_context(
                    tc.For_i(0, repeat, 1, hint_engines=(mybir.EngineType.PE,))
                )

            # gate accumulators [k-chunk, n] — 4 full PSUM banks
            psg = [
                psG.tile([128, nc_n], F32, tag=f"g{kc}", name=f"psg{kc}")
                for kc in range(4)
            ]

            # hypernetwork MLP (PSUM scratch borrowed from psg banks; the
            # later start=True bias matmuls reset them for accumulation)
            nc.tensor.matmul(psg[0][:], w1T[:], memT[:], start=True, stop=True)
            h1T = ep.tile([128, nc_n], F32R, tag="h1T")
            nc.scalar.activation(h1T[:], psg[0][:], AF.Tanh, bias=b1c[:])
            nc.tensor.matmul(psg[1][0:BOT, :], w2T[:], h1T[:], start=True, stop=True)
            hT2 = ep.tile([128, nc_n], F32R, tag="hT2")
            nc.scalar.activation(hT2[0:BOT, :], psg[1][0:BOT, :], AF.Tanh, bias=b2c[:])
            nc.scalar.activation(hT2[BOT:128, :], psg[1][0:BOT, :], AF.Tanh, bias=b2c[:])

            # fold the b3 term in first (start=True resets the banks)
            for kc in range(4):
                ks = slice(kc * 128, kc * 128 + 128)
                nc.tensor.matmul(psg[kc][:], b3a[:, ks], dT0[:], start=True, stop=False)
                nc.tensor.matmul(psg[kc][:], b3b[:, ks], dT1[:], start=False, stop=False)

            # main contraction, blocks of BLK pair-units.
            # Pair schedule PAIR_ORDER rotates the four 32-row strips so
            # back-to-back K=2 broadcast matmuls land on different PE row
            # groups (concurrent row tiles); host permutes w3 to match.
            NB = KU // BLK
            ND = KU // DBLK
            op_q = {}
            w3_q = {}

            def pair_src(t):
                # scheduled pair t reads dataT rows (2t, 2t+1)
                if t < 32:
                    return dT0, 2 * t
                return dT1, 2 * (t - 32)

            def emit_w3_dma(d):
                w3blk = w3pool.tile(
                    [128, DBLK, 2, K4] if USE_FP8 else [128, DBLK, 2 * K4],
                    FP8 if USE_FP8 else BF16, tag="w3sb", name="w3sb")
                srcw = w3p_d[:, d * DBLK * 2 * K4:(d + 1) * DBLK * 2 * K4]
                if USE_FP8:
                    srcw = srcw.rearrange("p (b h k) -> p b h k", b=DBLK, h=2)
                else:
                    srcw = srcw.rearrange("p (b k) -> p b k", b=DBLK)
                dq = nc.sync if d % 2 == 0 else nc.scalar
                dq.dma_start(w3blk[:], srcw)
                for j in range(DBLK):
                    w3_q[DBLK * d + j] = w3blk[:, j]

            op_const = None
            if SKIP_MULS or SKIP_PA:
                op_const = const.tile([128, 2, nc_n], FP8 if USE_FP8 else BF16,
                                      name="op_const")
                nc.vector.tensor_copy(
                    op_const[:],
                    memT[:, None, :].broadcast_to([128, 2, nc_n]))

            def emit_stage_a(b):
                us = [BLK * b + j for j in range(BLK)]
                pas = []
                if SKIP_PA:
                    for u in us:
                        op_q[u] = op_const
                    return
                for u in us:
                    pa2 = psA.tile([128, 2 * nc_n], F32, tag="A", name="pa2")
                    pas.append(pa2)
                    for h in range(2):
                        t = PAIR_ORDER[2 * u + h]
                        srct, base = pair_src(t)
                        sb = base // 32 * 32      # aligned strip base
                        w = (base - sb) // 2 * 128  # selector window
                        nc.tensor.matmul(
                            pa2[:, h * nc_n:(h + 1) * nc_n],
                            ppat[sb:sb + 32, w:w + 128],
                            srct[sb:sb + 32, :],
                            start=True, stop=True,
                            tile_position=(sb, 0),
                        )
                if SKIP_MULS:
                    for u in us:
                        op_q[u] = op_const
                    return
                for u, pa2 in zip(us, pas):
                    op2 = opool.tile([128, 2, nc_n], FP8 if USE_FP8 else BF16,
                                     tag="opT", name="op2")
                    nc.vector.tensor_mul(
                        op2[:],
                        pa2[:].rearrange("p (h n) -> p h n", h=2),
                        hT2[:, None, :].broadcast_to([128, 2, nc_n]),
                    )
                    op_q[u] = op2

            def emit_gates(b):
                for u in [BLK * b + j for j in range(BLK)]:
                    last = u == KU - 1
                    if SKIP_GATES:
                        w3_q[u] = None
                        if not (SKIP_MULS or SKIP_PA):
                            op_q[u] = None
                        continue
                    if USE_FP8:
                        for kc in range(4):
                            nc.tensor.matmul(
                                psg[kc][:],
                                w3_q[u][:, :, kc * 128:kc * 128 + 128],
                                op_q[u][:],
                                start=False, stop=last,
                                perf_mode=DRMODE,
                            )
                    else:
                        for h in range(2):
                            for kc in range(4):
                                nc.tensor.matmul(
                                    psg[kc][:],
                                    w3_q[u][:, h * K4 + kc * 128:h * K4 + kc * 128 + 128],
                                    op_q[u][:, h, :],
                                    start=False, stop=last and h == 1,
                                )
                    w3_q[u] = op_q[u] = None

            la_b = LAB_F(nc_n)
            BPD = DBLK // BLK          # blocks per w3 DMA
            LA_D = (la_b + 2 * BPD - 1) // BPD  # dma-block lookahead
            for d in range(min(LA_D, ND)):
                emit_w3_dma(d)
            for b in range(min(la_b, NB)):
                emit_stage_a(b)
            for b in range(NB):
                if b % BPD == 0 and b // BPD + LA_D < ND:
                    emit_w3_dma(b // BPD + LA_D)
                if b + la_b < NB:
                    emit_stage_a(b + la_b)
                emit_gates(b)

            # LSTM epilogue on [hid, n] tiles; k-chunk order: i, f, g, o.
            # Processed in two half-n chunks so the ACT chain of one half
            # overlaps the DVE chain of the other (shorter serial tail).
            g = [ep.tile([128, nc_n], F32, tag=f"gs{kc}", name=f"gs{kc}")
                 for kc in range(4)]
            i_t = ep.tile([128, nc_n], F32, tag="i_t")
            f_t = ep.tile([128, nc_n], F32, tag="f_t")
            g_t = ep.tile([128, nc_n], F32, tag="g_t")
            o_t = ep.tile([128, nc_n], F32, tag="o_t")
            t1 = ep.tile([128, nc_n], F32, tag="t1")
            t2 = ep.tile([128, nc_n], F32, tag="t2")
            cy = ep.tile([128, nc_n], F32, tag="cy")
            tcy = ep.tile([128, nc_n], F32, tag="tcy")
            hy = ep.tile([128, nc_n], F32, tag="hy")
            hn = nc_n // 2
            for c0, c1 in ((0, hn), (hn, nc_n)):
                cs = slice(c0, c1)
                for kc in range(4):
                    nc.scalar.activation(g[kc][:, cs], psg[kc][:, cs],
                                         AF.Sigmoid, bias=lb[:, kc:kc + 1],
                                         scale=Z_DESCALE if USE_FP8 else 1.0)
                nc.scalar.activation(i_t[:, cs], g[0][:, cs], AF.Sigmoid)
                nc.scalar.activation(f_t[:, cs], g[1][:, cs], AF.Sigmoid)
                nc.scalar.activation(g_t[:, cs], g[2][:, cs], AF.Tanh)
                nc.scalar.activation(o_t[:, cs], g[3][:, cs], AF.Sigmoid)
                nc.vector.tensor_mul(t1[:, cs], cxT[:, cs], f_t[:, cs])
                nc.vector.tensor_mul(t2[:, cs], i_t[:, cs], g_t[:, cs])
                nc.vector.tensor_add(cy[:, cs], t1[:, cs], t2[:, cs])
                nc.scalar.activation(tcy[:, cs], cy[:, cs], AF.Tanh)
                nc.vector.tensor_mul(hy[:, cs], o_t[:, cs], tcy[:, cs])
                nc.sync.dma_start(cyT_o[:, cs], cy[:, cs])
                nc.scalar.dma_start(hyT_o[:, cs], hy[:, cs])

            loop_ctx.close()

    nc.finalize()
    return nc


def kernel(x, index, hx, cx, emb_table, w1, b1, w2, b2, w3, b3, lstm_bias,
           _trace=False):
    x = np.asarray(x, np.float32)
    index = np.asarray(index)
    hx = np.asarray(hx, np.float32)
    cx = np.asarray(cx, np.float32)
    emb_table = np.asarray(emb_table, np.float32)
    w1 = np.asarray(w1, np.float32)
    b1 = np.asarray(b1, np.float32)
    w2 = np.asarray(w2, np.float32)
    b2 = np.asarray(b2, np.float32)
    w3 = np.asarray(w3, np.float32)
    b3 = np.asarray(b3, np.float32)
    lstm_bias = np.asarray(lstm_bias, np.float32)

    nc_n = N // NCORES
    if ("nc", NCORES) not in _cache:
        _cache[("nc", NCORES)] = _build_program(nc_n=nc_n)
    nc = _cache[("nc", NCORES)]

    # host-side input prep (sharding + layout)
    mem = emb_table[index]                                   # (N, EMB)
    c = np.ascontiguousarray
    w1T = c(w1.T)
    w2T = c(w2.T)
    b1c = b1.reshape(EMB, 1)
    b2c = b2.reshape(BOT, 1)
    # W3perm[(d*64+b), k] = w3[d*512+k, b]; then pair K-tiles (2u, 2u+1)
    w3perm = w3.reshape(D, K4, BOT).transpose(0, 2, 1).reshape(D * BOT, K4)
    # [128, KU, 2, K4]: K-tiles permuted to PAIR_ORDER, units along free dim
    w3tiles = w3perm.reshape(KT, 128, K4)[PAIR_ORDER]
    w3pair = w3tiles.reshape(KU, 2, 128, K4).transpose(2, 0, 1, 3)
    if USE_FP8:
        w3p = c((w3pair * W3_SCALE).reshape(128, KU * 2 * K4)).astype(
            ml_dtypes.float8_e4m3)
        zs = OP_SCALE * W3_SCALE
    else:
        w3p = c(w3pair.reshape(128, KU * 2 * K4)).astype(ml_dtypes.bfloat16)
        zs = 1.0
    b3r = b3.reshape(D, K4) * zs
    b3a = c(b3r[0:INPUT])
    b3b = c(b3r[INPUT:D])
    lbias = c(lstm_bias.reshape(4, HID).T)                   # [j, kc]
    # K=32 strip-broadcast selector: window o (even row offset within a
    # 32-row strip) picks rows (o, o+1) -> out cols (0:64, 64:128). The
    # matmul reads a full aligned strip (BIR requires 32-aligned partition
    # bases); the one-hot stationary selects the pair.
    ppat = np.zeros((128, 16 * 128), np.float32)
    pv = OP_SCALE if USE_FP8 else 1.0
    for o in range(0, 32, 2):
        w = o // 2 * 128
        ppat[o::32, w:w + 64] = pv
        ppat[o + 1::32, w + 64:w + 128] = pv

    in_maps = []
    for ci in range(NCORES):
        sl = slice(ci * nc_n, (ci + 1) * nc_n)
        in_maps.append({
            "memT": c(mem[sl].T),
            "dT0": c(x[sl].T),
            "dT1": c(hx[sl].T),
            "cxT": c(cx[sl].T),
            "w1T": w1T, "w2T": w2T, "b1c": b1c, "b2c": b2c,
            "w3p": w3p, "b3a": b3a, "b3b": b3b,
            "lbias": lbias, "ppat": ppat,
        })

    res = run_bass_kernel_spmd(nc, in_maps, list(range(NCORES)), trace=_trace)
    hy = np.concatenate([r["hyT"].T for r in res.results], axis=0)
    cy = np.concatenate([r["cyT"].T for r in res.results], axis=0)
    if _trace:
        kernel.last_results = res
    return hy.astype(np.float32), cy.astype(np.float32)
